# revision 1
# baseline (speedup 1.0000x reference)
"""Trainium2 Bass kernel for nn_AttentionHead.

Computation (per batch b):
    Q = Wq @ x_b, K = Wk @ x_b, V = Wv @ x_b        (x_b: [C=256, N=4096])
    S = Q^T K   [N, N];  A = softmax_k(S)
    out_b = V @ A^T                                  ([VC=128, N])

Sharding: 8 cores = 4 batches x 2 query-halves. Each core computes K/V^T for
its full batch and Q for its 2048-query half; a flash-style loop over 32 key
chunks of 128 never materializes the full [4096, 4096] affinity.

Numerics: QK logits in fp32r (full PE rate, near-fp32 accuracy pre-exp);
exp tiles and V^T in bf16 (linear path, errors stay ~0.3%). Softmax
denominators: exp tiles are tree-summed pairwise on VectorE down to one
[128, QT] partial per query-half; the final 128-way reduction and the
normalization happen on the host during unshard.
"""

import numpy as np

B, C, VC, H, W = 4, 256, 128, 64, 64
N = H * W            # keys per batch
MQ = N // 2          # queries per core
QT = 1024            # query tile (PSUM-sized)
KC = N // 128        # key chunks of 128

_cached_nc = None


def _build():
    from contextlib import ExitStack

    import concourse.bacc as bacc
    import concourse.mybir as mybir
    import concourse.tile as tile

    f32 = mybir.dt.float32
    f32r = mybir.dt.float32r
    bf16 = mybir.dt.bfloat16
    Exp = mybir.ActivationFunctionType.Exp

    nc = bacc.Bacc("TRN2", target_bir_lowering=False, debug=False, num_devices=8)

    xk_d = nc.dram_tensor("xk", [C, N], f32r, kind="ExternalInput")
    xq_d = nc.dram_tensor("xq", [C, MQ], f32r, kind="ExternalInput")
    w_d = {
        "wq": nc.dram_tensor("wq", [C, VC], f32r, kind="ExternalInput"),
        "wk": nc.dram_tensor("wk", [C, VC], f32r, kind="ExternalInput"),
        "wv": nc.dram_tensor("wv", [C, VC], f32r, kind="ExternalInput"),
    }
    oc_d = nc.dram_tensor("oc", [2, 128, QT], f32, kind="ExternalOutput")
    oss_d = nc.dram_tensor("oss", [2, 128, QT], bf16, kind="ExternalOutput")

    with tile.TileContext(nc) as tc, ExitStack() as ctx:
        persist = ctx.enter_context(tc.tile_pool(name="persist", bufs=1))
        wpool = ctx.enter_context(tc.tile_pool(name="w", bufs=1))
        xp = ctx.enter_context(tc.tile_pool(name="xp", bufs=1))

        wts = {}
        for nm in ("wq", "wk", "wv"):
            for cc in range(2):
                t = wpool.tile([128, VC], f32r, tag=f"{nm}{cc}")
                nc.gpsimd.dma_start(t[:], w_d[nm][cc * 128 : (cc + 1) * 128, :])
                wts[(nm, cc)] = t

        K_t = persist.tile([128, N], f32r, tag="K")
        Q_t = persist.tile([128, MQ], f32r, tag="Q")
        VT = persist.tile([128, KC * 128], bf16, tag="VT")

        xk_t = [
            xp.tile([128, N], f32r, tag=f"xk{cc}", name=f"xk{cc}") for cc in range(2)
        ]
        xq_t = [
            xp.tile([128, MQ], f32r, tag=f"xq{cc}", name=f"xq{cc}") for cc in range(2)
        ]
        # piece order matches consumption: the first 512 columns of xq/xk
        # (Q tile 0, K tile 0, first V^T blocks) gate the first attention
        # chunk, so they go as small pieces first. A depth-4 dependency
        # window keeps ~2MB in flight so early pieces finish early instead
        # of all pieces finishing together.
        from concourse.tile_rust import add_dep_helper

        _dmas = []

        def dma_piece(xt, xd, c0, c1, eng):
            for cc in range(2):
                ins = eng.dma_start(
                    xt[cc][:, c0:c1],
                    xd[cc * 128 : (cc + 1) * 128, c0:c1],
                )
                if len(_dmas) >= 4:
                    add_dep_helper(
                        ins.ins, _dmas[len(_dmas) - 4].ins, reason="dma window"
                    )
                _dmas.append(ins)

        dma_piece(xq_t, xq_d, 0, 512, nc.sync)
        dma_piece(xk_t, xk_d, 0, 512, nc.sync)
        dma_piece(xq_t, xq_d, 512, 1024, nc.sync)
        dma_piece(xk_t, xk_d, 512, 1024, nc.sync)
        dma_piece(xk_t, xk_d, 1024, 2048, nc.sync)
        dma_piece(xq_t, xq_d, 1024, 2048, nc.gpsimd)
        dma_piece(xk_t, xk_d, 2048, 3072, nc.gpsimd)
        dma_piece(xk_t, xk_d, 3072, 4096, nc.gpsimd)

        def emit_proj_tile(pool, dst, wnm, xt, t):
            ps = pool.tile([128, 512], f32, tag="projps", name="ps")
            for cc in range(2):
                nc.tensor.matmul(
                    ps[:],
                    wts[(wnm, cc)][:],
                    xt[cc][:, t * 512 : (t + 1) * 512],
                    start=(cc == 0),
                    stop=(cc == 1),
                )
            nc.vector.tensor_copy(dst[:, t * 512 : (t + 1) * 512], ps[:])

        def emit_vt(pool, j):
            # V^T block j: [n-block, d] = x_block.T @ Wv.T
            tp = pool.tile([128, 512], f32, tag="projps", name="tp")
            for cc in range(2):
                nc.tensor.matmul(
                    tp[:, 0:128],
                    xk_t[cc][:, j * 128 : (j + 1) * 128],
                    wts[("wv", cc)][:],
                    start=(cc == 0),
                    stop=(cc == 1),
                )
            nc.vector.tensor_copy(VT[:, j * 128 : (j + 1) * 128], tp[:, 0:128])

        XQ0, P0, P1, XQ1, P2, P3 = 0.0105, 0.013, 0.016, 0.0185, 0.021, 0.0235
        K_FLOOR = {0: P0, 1: P0, 2: P1, 3: P1, 4: P2, 5: P2, 6: P3, 7: P3}

        def vt_floor(j):
            return [P0, P1, P2, P3][j // 8]

        spool = ctx.enter_context(tc.tile_pool(name="spool", bufs=2, space="PSUM"))
        pcpool = ctx.enter_context(tc.tile_pool(name="pcpool", bufs=1, space="PSUM"))

        with tc.tile_pool(name="projps", bufs=2, space="PSUM") as pps:
            for t in range(2):
                with tc.tile_wait_until(XQ0):
                    emit_proj_tile(pps, Q_t, "wq", xq_t, t)
            with tc.tile_wait_until(P0):
                emit_proj_tile(pps, K_t, "wk", xk_t, 0)
                for j in range(2):
                    emit_vt(pps, j)

        with (
            tc.tile_pool(name="lzps", bufs=2, space="PSUM") as lzps,
            tc.tile_pool(name="epool", bufs=8) as epool,
            tc.tile_pool(name="treep", bufs=3) as treep,
            tc.tile_pool(name="opool", bufs=2) as opool,
        ):
            pairs = [(qt, j) for qt in range(2) for j in range(KC)]
            ps_tiles = {}

            def emit_qk(qt, j):
                ps = spool.tile([128, QT], f32, tag="ps", name="ps")
                for qq in range(2):
                    nc.tensor.matmul(
                        ps[:, qq * 512 : (qq + 1) * 512],
                        K_t[:, j * 128 : (j + 1) * 128],
                        Q_t[:, qt * QT + qq * 512 : qt * QT + (qq + 1) * 512],
                        start=True,
                        stop=True,
                    )
                ps_tiles[(qt, j)] = ps

            # binary-counter pairwise reduction of exp tiles on DVE
            pending = []

            def tree_push(t, level=0):
                while pending and pending[-1][0] == level:
                    _, other = pending.pop()
                    nt = treep.tile(
                        [128, QT], bf16, tag=f"l{level + 1}", name=f"tl{level + 1}"
                    )
                    nc.vector.tensor_add(nt[:], other[:], t[:])
                    t, level = nt, level + 1
                pending.append((level, t))

            pc = None
            emit_qk(*pairs[0])
            for i, (qt, j) in enumerate(pairs):
                if i + 1 < len(pairs):
                    emit_qk(*pairs[i + 1])
                if 1 <= i <= 7:
                    with tc.tile_wait_until(K_FLOOR[i]):
                        emit_proj_tile(lzps, K_t, "wk", xk_t, i)
                if 8 <= i <= 9:
                    with tc.tile_wait_until(XQ1):
                        emit_proj_tile(lzps, Q_t, "wq", xq_t, i - 6)
                if qt == 0 and j + 2 < KC:
                    with tc.tile_wait_until(vt_floor(j + 2)):
                        emit_vt(lzps, j + 2)
                if j == 0:
                    pc = pcpool.tile([128, QT], f32, tag="pc", name="pc")
                ps = ps_tiles.pop((qt, j))
                es = epool.tile([128, QT], bf16, tag="es", name="es")
                nc.scalar.activation(es[:], ps[:], Exp)
                first, last = j == 0, j == KC - 1
                for qq in range(2):
                    sl = slice(qq * 512, (qq + 1) * 512)
                    nc.tensor.matmul(
                        pc[:, sl],
                        VT[:, j * 128 : (j + 1) * 128],
                        es[:, sl],
                        start=first,
                        stop=last,
                    )
                tree_push(es)
                if last:
                    acc = pending.pop()[1]
                    pending.clear()
                    so = opool.tile([128, QT], f32, tag="so", name="so")
                    for qq in range(2):
                        sl = slice(qq * 512, (qq + 1) * 512)
                        nc.vector.tensor_copy(so[:, sl], pc[:, sl])
                        nc.sync.dma_start(oc_d[qt, :, sl], so[:, sl])
                        nc.sync.dma_start(oss_d[qt, :, sl], acc[:, sl])

    nc.compile()
    return nc


def make_in_maps(x, Wq, Wk, Wv):
    x = np.ascontiguousarray(np.asarray(x, dtype=np.float32).reshape(B, C, N))
    wt = {
        "wq": np.ascontiguousarray(np.asarray(Wq, dtype=np.float32).T),
        "wk": np.ascontiguousarray(np.asarray(Wk, dtype=np.float32).T),
        "wv": np.ascontiguousarray(np.asarray(Wv, dtype=np.float32).T),
    }

    in_maps = []
    for core in range(8):
        b, h = core // 2, core % 2
        in_maps.append(
            {
                "xk": x[b],
                "xq": np.ascontiguousarray(x[b][:, h * MQ : (h + 1) * MQ]),
                **wt,
            }
        )
    return in_maps


def assemble_output(results):
    out = np.empty((B, VC, N), dtype=np.float32)
    for core, r in enumerate(results):
        b, h = core // 2, core % 2
        sums = r["oss"].astype(np.float32).sum(axis=1, keepdims=True)  # [2,1,QT]
        core_out = r["oc"] / sums                                     # [2,128,QT]
        out[b, :, h * MQ : (h + 1) * MQ] = np.concatenate(
            [core_out[0], core_out[1]], axis=1
        )
    return out.reshape(B, VC, H, W)


def _results_sane(results):
    for r in results:
        oc, oss = r["oc"], np.asarray(r["oss"], dtype=np.float32)
        if not (np.isfinite(oc).all() and np.isfinite(oss).all()):
            return False
        if oss.sum(axis=1).min() <= 0.0:      # softmax denominators
            return False
    return True


def kernel(x, Wq, Wk, Wv):
    global _cached_nc
    from concourse.bass_utils import run_bass_kernel_spmd

    if _cached_nc is None:
        _cached_nc = _build()
    in_maps = make_in_maps(x, Wq, Wk, Wv)
    results = None
    for attempt in range(3):
        try:
            res = run_bass_kernel_spmd(
                _cached_nc, in_maps, core_ids=list(range(8))
            )
        except Exception:
            if attempt == 2:
                raise
            continue
        results = res.results
        if _results_sane(results):
            break
    return assemble_output(results)



# revision 2
# speedup vs baseline: 1.0599x; 1.0599x over previous
"""Trainium2 Bass kernel for nn_AttentionHead.

Computation (per batch b):
    Q = Wq @ x_b, K = Wk @ x_b, V = Wv @ x_b        (x_b: [C=256, N=4096])
    S = Q^T K   [N, N];  A = softmax_k(S)
    out_b = V @ A^T                                  ([VC=128, N])

Sharding: 8 cores = 4 batches x 2 query-halves. Each core computes K/V^T for
its full batch and Q for its 2048-query half; a flash-style loop over 32 key
chunks of 128 never materializes the full [4096, 4096] affinity.

Numerics: the host casts x and the weights to bf16, so every matmul runs the
full-rate bf16 path (QK logits accumulate in fp32 PSUM pre-exp). Softmax
denominators: exp tiles are summed on DVE in 2048-wide grouped chains down to
4 partials per query-half; the final reduction and the normalization happen
on the host during unshard.
"""

import numpy as np

B, C, VC, H, W = 4, 256, 128, 64, 64
N = H * W            # keys per batch
MQ = N // 2          # queries per core
QT = 1024            # query tile (PSUM-sized)
KC = N // 128        # key chunks of 128
NG = 4               # softmax partial-sum groups per query tile

_cached_nc = None


def _build():
    from contextlib import ExitStack

    import concourse.bacc as bacc
    import concourse.mybir as mybir
    import concourse.tile as tile

    f32 = mybir.dt.float32
    bf16 = mybir.dt.bfloat16
    Exp = mybir.ActivationFunctionType.Exp

    nc = bacc.Bacc("TRN2", target_bir_lowering=False, debug=False, num_devices=8)

    xk_d = nc.dram_tensor("xk", [C, N], bf16, kind="ExternalInput")
    xq_d = nc.dram_tensor("xq", [C, MQ], bf16, kind="ExternalInput")
    w_d = {
        "wq": nc.dram_tensor("wq", [C, VC], bf16, kind="ExternalInput"),
        "wk": nc.dram_tensor("wk", [C, VC], bf16, kind="ExternalInput"),
        "wv": nc.dram_tensor("wv", [C, VC], bf16, kind="ExternalInput"),
    }
    oc_d = nc.dram_tensor("oc", [2, 128, QT], f32, kind="ExternalOutput")
    oss_d = nc.dram_tensor("oss", [2, NG, 128, QT], bf16, kind="ExternalOutput")

    with tile.TileContext(nc) as tc, ExitStack() as ctx:
        persist = ctx.enter_context(tc.tile_pool(name="persist", bufs=1))
        wpool = ctx.enter_context(tc.tile_pool(name="w", bufs=1))
        xp = ctx.enter_context(tc.tile_pool(name="xp", bufs=1))

        wts = {}
        for nm in ("wq", "wk", "wv"):
            for cc in range(2):
                t = wpool.tile([128, VC], bf16, tag=f"{nm}{cc}")
                nc.sync.dma_start(t[:], w_d[nm][cc * 128 : (cc + 1) * 128, :])
                wts[(nm, cc)] = t

        K_t = persist.tile([128, N], bf16, tag="K")
        Q_t = persist.tile([128, MQ], bf16, tag="Q")
        VT = persist.tile([128, KC * 128], bf16, tag="VT")

        xk_t = [
            xp.tile([128, N], bf16, tag=f"xk{cc}", name=f"xk{cc}") for cc in range(2)
        ]
        xq_t = [
            xp.tile([128, MQ], bf16, tag=f"xq{cc}", name=f"xq{cc}") for cc in range(2)
        ]
        # piece order matches consumption: xq first (Q proj gates the first
        # QK), then xk pieces in key order. A small dependency window keeps
        # a couple of pieces in flight so early pieces finish early.
        from concourse.tile_rust import add_dep_helper

        _dmas = []

        def dma_piece(xt, xd, c0, c1, eng):
            for cc in range(2):
                ins = eng.dma_start(
                    xt[cc][:, c0:c1],
                    xd[cc * 128 : (cc + 1) * 128, c0:c1],
                )
                if len(_dmas) >= 4:
                    add_dep_helper(
                        ins.ins, _dmas[len(_dmas) - 4].ins, reason="dma window"
                    )
                _dmas.append(ins)

        dma_piece(xq_t, xq_d, 0, 1024, nc.sync)
        dma_piece(xk_t, xk_d, 0, 512, nc.sync)
        dma_piece(xk_t, xk_d, 512, 1024, nc.gpsimd)
        dma_piece(xq_t, xq_d, 1024, 2048, nc.sync)
        dma_piece(xk_t, xk_d, 1024, 2048, nc.gpsimd)
        dma_piece(xk_t, xk_d, 2048, 3072, nc.sync)
        dma_piece(xk_t, xk_d, 3072, 4096, nc.gpsimd)

        def emit_proj_tile(pool, dst, wnm, xt, t):
            ps = pool.tile([128, 512], f32, tag="projps", name="ps")
            for cc in range(2):
                nc.tensor.matmul(
                    ps[:],
                    wts[(wnm, cc)][:],
                    xt[cc][:, t * 512 : (t + 1) * 512],
                    start=(cc == 0),
                    stop=(cc == 1),
                )
            nc.vector.tensor_copy(dst[:, t * 512 : (t + 1) * 512], ps[:])

        def emit_vt_quad(pool, q):
            # V^T blocks 4q..4q+3: each [n=128, d=128] = x_block.T @ Wv.T;
            # four blocks share one PSUM tile so one wide copy drains them.
            tp = pool.tile([128, 512], f32, tag="projps", name="tp")
            for jj in range(4):
                j = 4 * q + jj
                for cc in range(2):
                    nc.tensor.matmul(
                        tp[:, jj * 128 : (jj + 1) * 128],
                        xk_t[cc][:, j * 128 : (j + 1) * 128],
                        wts[("wv", cc)][:],
                        start=(cc == 0),
                        stop=(cc == 1),
                    )
            nc.vector.tensor_copy(VT[:, q * 512 : (q + 1) * 512], tp[:])

        # Floors (ms): don't emit projection work before its DMA piece can
        # have landed, so the PE queue never blocks on a DMA semaphore.
        XQ0, XK0, XK1, XQ1, XK2, XK3, XK4 = (
            0.0080, 0.0090, 0.0105, 0.0115, 0.0130, 0.0148, 0.0166,
        )
        K_FLOOR = {1: XK1, 2: XK2, 3: XK2, 4: XK3, 5: XK3, 6: XK4, 7: XK4}
        VQ_FLOOR = {1: XK1, 2: XK2, 3: XK2, 4: XK3, 5: XK3, 6: XK4, 7: XK4}

        spool = ctx.enter_context(tc.tile_pool(name="spool", bufs=2, space="PSUM"))
        pcpool = ctx.enter_context(tc.tile_pool(name="pcpool", bufs=1, space="PSUM"))

        with tc.tile_pool(name="projps", bufs=2, space="PSUM") as pps:
            for t in range(2):
                with tc.tile_wait_until(XQ0):
                    emit_proj_tile(pps, Q_t, "wq", xq_t, t)
            with tc.tile_wait_until(XK0):
                emit_proj_tile(pps, K_t, "wk", xk_t, 0)
                emit_vt_quad(pps, 0)

        with (
            tc.tile_pool(name="lzps", bufs=2, space="PSUM") as lzps,
            tc.tile_pool(name="epool", bufs=4) as epool,
            tc.tile_pool(name="treep", bufs=2) as treep,
            tc.tile_pool(name="fold", bufs=2) as foldp,
            tc.tile_pool(name="opool", bufs=2) as opool,
        ):
            pairs = [(qt, j) for qt in range(2) for j in range(KC)]
            ps_tiles = {}

            def emit_qk(qt, j):
                ps = spool.tile([128, QT], f32, tag="ps", name="ps")
                for qq in range(2):
                    nc.tensor.matmul(
                        ps[:, qq * 512 : (qq + 1) * 512],
                        K_t[:, j * 128 : (j + 1) * 128],
                        Q_t[:, qt * QT + qq * 512 : qt * QT + (qq + 1) * 512],
                        start=True,
                        stop=True,
                    )
                ps_tiles[(qt, j)] = ps

            pc = None
            es_dt = None
            acc = None
            emit_qk(*pairs[0])
            for i, (qt, j) in enumerate(pairs):
                if i + 1 < len(pairs):
                    emit_qk(*pairs[i + 1])
                # interleave remaining projections into the early iterations
                if qt == 0 and 1 <= j <= 14:
                    if j % 2 == 1:
                        t = (j + 1) // 2
                        with tc.tile_wait_until(K_FLOOR[t]):
                            emit_proj_tile(lzps, K_t, "wk", xk_t, t)
                    else:
                        q = j // 2
                        with tc.tile_wait_until(VQ_FLOOR[q]):
                            emit_vt_quad(lzps, q)
                if qt == 0 and 15 <= j <= 16:
                    with tc.tile_wait_until(XQ1):
                        emit_proj_tile(lzps, Q_t, "wq", xq_t, j - 13)
                if j == 0:
                    pc = pcpool.tile([128, QT], f32, tag="pc", name="pc")
                ps = ps_tiles.pop((qt, j))
                if j % 2 == 0:
                    es_dt = epool.tile([128, 2 * QT], bf16, tag="es", name="es")
                es = es_dt[:, (j % 2) * QT : (j % 2 + 1) * QT]
                nc.scalar.activation(es, ps[:], Exp)
                first, last = j == 0, j == KC - 1
                for qq in range(2):
                    sl = slice(qq * 512, (qq + 1) * 512)
                    nc.tensor.matmul(
                        pc[:, sl],
                        VT[:, j * 128 : (j + 1) * 128],
                        es[:, qq * 512 : (qq + 1) * 512],
                        start=first,
                        stop=last,
                    )
                # grouped softmax partial sums: 4 groups of 4 double-tiles;
                # wide [128, 2048] chained adds, then a fold to [128, QT]
                # that ships to the host for the cross-partition finish.
                if j % 2 == 1:
                    g, m = j // 8, (j % 8) // 2
                    if m == 0:
                        acc = es_dt
                    else:
                        if m == 1:
                            nacc = treep.tile(
                                [128, 2 * QT], bf16, tag="acc", name="acc"
                            )
                            nc.vector.tensor_add(nacc[:], acc[:], es_dt[:])
                            acc = nacc
                        else:
                            nc.vector.tensor_add(acc[:], acc[:], es_dt[:])
                        if m == 3:
                            fo = foldp.tile([128, QT], bf16, tag="fo", name="fo")
                            nc.vector.tensor_add(
                                fo[:], acc[:, 0:QT], acc[:, QT : 2 * QT]
                            )
                            nc.sync.dma_start(oss_d[qt, g], fo[:])
                if last:
                    so = opool.tile([128, QT], f32, tag="so", name="so")
                    for qq in range(2):
                        sl = slice(qq * 512, (qq + 1) * 512)
                        nc.vector.tensor_copy(so[:, sl], pc[:, sl])
                        nc.sync.dma_start(oc_d[qt, :, sl], so[:, sl])

    nc.compile()
    return nc


def make_in_maps(x, Wq, Wk, Wv):
    import ml_dtypes

    bf16 = ml_dtypes.bfloat16
    x = np.ascontiguousarray(
        np.asarray(x, dtype=np.float32).reshape(B, C, N).astype(bf16)
    )
    wt = {
        "wq": np.ascontiguousarray(np.asarray(Wq, dtype=np.float32).T.astype(bf16)),
        "wk": np.ascontiguousarray(np.asarray(Wk, dtype=np.float32).T.astype(bf16)),
        "wv": np.ascontiguousarray(np.asarray(Wv, dtype=np.float32).T.astype(bf16)),
    }

    in_maps = []
    for core in range(8):
        b, h = core // 2, core % 2
        in_maps.append(
            {
                "xk": x[b],
                "xq": np.ascontiguousarray(x[b][:, h * MQ : (h + 1) * MQ]),
                **wt,
            }
        )
    return in_maps


def assemble_output(results):
    out = np.empty((B, VC, N), dtype=np.float32)
    for core, r in enumerate(results):
        b, h = core // 2, core % 2
        # oss: [2, NG, 128, QT] partial sums; reduce groups+partitions
        sums = r["oss"].astype(np.float32).sum(axis=(1, 2))[:, None, :]  # [2,1,QT]
        core_out = r["oc"] / sums                                        # [2,128,QT]
        out[b, :, h * MQ : (h + 1) * MQ] = np.concatenate(
            [core_out[0], core_out[1]], axis=1
        )
    return out.reshape(B, VC, H, W)


def _results_sane(results):
    for r in results:
        oc, oss = r["oc"], np.asarray(r["oss"], dtype=np.float32)
        if not (np.isfinite(oc).all() and np.isfinite(oss).all()):
            return False
        if oss.sum(axis=(1, 2)).min() <= 0.0:      # softmax denominators
            return False
    return True


def kernel(x, Wq, Wk, Wv):
    global _cached_nc
    from concourse.bass_utils import run_bass_kernel_spmd

    if _cached_nc is None:
        _cached_nc = _build()
    in_maps = make_in_maps(x, Wq, Wk, Wv)
    results = None
    for attempt in range(3):
        try:
            res = run_bass_kernel_spmd(
                _cached_nc, in_maps, core_ids=list(range(8))
            )
        except Exception:
            if attempt == 2:
                raise
            continue
        results = res.results
        if _results_sane(results):
            break
    return assemble_output(results)


# revision 7
# speedup vs baseline: 1.0973x; 1.0353x over previous
"""Trainium2 Bass kernel for nn_AttentionHead.

Computation (per batch b):
    Q = Wq @ x_b, K = Wk @ x_b, V = Wv @ x_b        (x_b: [C=256, N=4096])
    S = Q^T K   [N, N];  A = softmax_k(S)
    out_b = V @ A^T                                  ([VC=128, N])

Sharding: 8 cores = 4 batches x 2 query-halves. Each core computes K/V^T for
its full batch and Q for its 2048-query half; a flash-style loop over 32 key
chunks of 128 never materializes the full [4096, 4096] affinity.

Numerics: the host casts x and the weights to fp16 (halves input DMA and
runs every matmul at the full-rate 16-bit PE path while keeping ~10 mantissa
bits through the logits, accumulated in fp32 PSUM). exp tiles are bf16 (fp16
would overflow: logits reach ~19 un-normalized). Softmax denominators: exp
tiles are summed on DVE in 2048-wide grouped chains down to 5 partials per
query-half; the final cross-partition reduction and the normalization happen
on the host during unshard.
"""

import numpy as np

B, C, VC, H, W = 4, 256, 128, 64, 64
N = H * W            # keys per batch
MQ = N // 2          # queries per core
QT = 1024            # query tile (PSUM-sized)
KC = N // 128        # key chunks of 128
NG = 5               # softmax partial-sum tiles per query tile

_cached_nc = None


def _build():
    from contextlib import ExitStack

    import concourse.bacc as bacc
    import concourse.mybir as mybir
    import concourse.tile as tile

    f32 = mybir.dt.float32
    f16 = mybir.dt.float16
    bf16 = mybir.dt.bfloat16
    Exp = mybir.ActivationFunctionType.Exp

    nc = bacc.Bacc("TRN2", target_bir_lowering=False, debug=False, num_devices=8)

    xk_d = nc.dram_tensor("xk", [C, N], f16, kind="ExternalInput")
    xq_d = nc.dram_tensor("xq", [C, MQ], f16, kind="ExternalInput")
    w_d = {
        "wq": nc.dram_tensor("wq", [C, VC], f16, kind="ExternalInput"),
        "wk": nc.dram_tensor("wk", [C, VC], f16, kind="ExternalInput"),
        "wv": nc.dram_tensor("wv", [C, VC], f16, kind="ExternalInput"),
    }
    oc_d = nc.dram_tensor("oc", [2, 128, QT], f32, kind="ExternalOutput")
    oss_d = nc.dram_tensor("oss", [2, NG, 128, QT], bf16, kind="ExternalOutput")

    with tile.TileContext(nc) as tc, ExitStack() as ctx:
        persist = ctx.enter_context(tc.tile_pool(name="persist", bufs=1))
        wpool = ctx.enter_context(tc.tile_pool(name="w", bufs=1))
        xp = ctx.enter_context(tc.tile_pool(name="xp", bufs=1))

        wts = {
            (nm, cc): wpool.tile(
                [128, VC], f16, tag=f"{nm}{cc}", name=f"{nm}{cc}"
            )
            for nm in ("wq", "wk", "wv")
            for cc in range(2)
        }

        K_t = persist.tile([128, N], f16, tag="K")
        Q_t = persist.tile([128, MQ], f16, tag="Q")
        VT = persist.tile([128, KC * 128], bf16, tag="VT")

        xk_t = [
            xp.tile([128, N], f16, tag=f"xk{cc}", name=f"xk{cc}") for cc in range(2)
        ]
        xq_t = [
            xp.tile([128, MQ], f16, tag=f"xq{cc}", name=f"xq{cc}") for cc in range(2)
        ]

        # Input DMA: four issue engines in parallel so the first pieces land
        # as early as possible after the framework preamble; per-engine
        # dependency windows keep at most ~2 pieces in flight per queue so
        # early pieces finish early.
        from concourse.tile_rust import add_dep_helper

        _dmas = {}

        def dma_piece(eng, ename, dst_t, src_d, c0, c1, p0=0, p1=256):
            lst = _dmas.setdefault(ename, [])
            for cc in range(2):
                if not (p0 <= cc * 128 < p1):
                    continue
                ins = eng.dma_start(
                    dst_t[cc][:, c0:c1],
                    src_d[cc * 128 : (cc + 1) * 128, c0:c1],
                )
                if len(lst) >= 4:
                    add_dep_helper(ins.ins, lst[-4].ins, reason="dma window")
                lst.append(ins)

        def dma_w(eng, ename, nm, cc):
            lst = _dmas.setdefault(ename, [])
            ins = eng.dma_start(
                wts[(nm, cc)][:], w_d[nm][cc * 128 : (cc + 1) * 128, :]
            )
            if len(lst) >= 4:
                add_dep_helper(ins.ins, lst[-4].ins, reason="dma window")
            lst.append(ins)

        # scalar: the QK-path weights (it idles until the first exp anyway);
        # sync/gpsimd stream x in consumption order.
        dma_w(nc.scalar, "a", "wq", 0)
        dma_w(nc.scalar, "a", "wk", 0)
        dma_w(nc.scalar, "a", "wq", 1)
        dma_w(nc.scalar, "a", "wk", 1)
        dma_piece(nc.gpsimd, "g", xk_t, xk_d, 0, 512)
        dma_w(nc.gpsimd, "g", "wv", 0)
        dma_w(nc.gpsimd, "g", "wv", 1)
        dma_piece(nc.sync, "s", xq_t, xq_d, 0, 512)
        dma_piece(nc.sync, "s", xq_t, xq_d, 512, 1024)
        dma_piece(nc.sync, "s", xk_t, xk_d, 512, 1024)
        dma_piece(nc.gpsimd, "g", xq_t, xq_d, 1024, 2048)
        dma_piece(nc.gpsimd, "g", xk_t, xk_d, 1024, 2048)
        dma_piece(nc.sync, "s", xk_t, xk_d, 2048, 3072)
        dma_piece(nc.gpsimd, "g", xk_t, xk_d, 3072, 4096)

        def emit_proj_tile(pool, dst, wnm, xt, t):
            ps = pool.tile([128, 512], f32, tag="projps", name="ps")
            for cc in range(2):
                nc.tensor.matmul(
                    ps[:],
                    wts[(wnm, cc)][:],
                    xt[cc][:, t * 512 : (t + 1) * 512],
                    start=(cc == 0),
                    stop=(cc == 1),
                )
            nc.vector.tensor_copy(dst[:, t * 512 : (t + 1) * 512], ps[:])

        def emit_vt_quad(pool, q):
            # V^T blocks 4q..4q+3: each [n=128, d=128] = x_block.T @ Wv.T;
            # four blocks share one PSUM tile so one wide copy drains them.
            tp = pool.tile([128, 512], f32, tag="projps", name="tp")
            for jj in range(4):
                j = 4 * q + jj
                for cc in range(2):
                    nc.tensor.matmul(
                        tp[:, jj * 128 : (jj + 1) * 128],
                        xk_t[cc][:, j * 128 : (j + 1) * 128],
                        wts[("wv", cc)][:],
                        start=(cc == 0),
                        stop=(cc == 1),
                    )
            nc.vector.tensor_copy(VT[:, q * 512 : (q + 1) * 512], tp[:])

        # Floors (ms): don't emit projection work before its DMA piece can
        # have landed, so the PE queue never blocks on a DMA semaphore.
        QF0, QF1, KF0, VF0 = 0.0072, 0.0084, 0.0088, 0.0094
        K_FLOOR = {1: 0.0100, 2: 0.0115, 3: 0.0123, 4: 0.0140, 5: 0.0148,
                   6: 0.0165, 7: 0.0173}
        V_FLOOR = {1: 0.0100, 2: 0.0119, 3: 0.0127, 4: 0.0144, 5: 0.0152,
                   6: 0.0169, 7: 0.0177}
        XQ1 = 0.0110

        spool = ctx.enter_context(tc.tile_pool(name="spool", bufs=2, space="PSUM"))
        pcpool = ctx.enter_context(tc.tile_pool(name="pcpool", bufs=1, space="PSUM"))

        with tc.tile_pool(name="projps", bufs=2, space="PSUM") as pps:
            with tc.tile_wait_until(QF0):
                emit_proj_tile(pps, Q_t, "wq", xq_t, 0)
            with tc.tile_wait_until(QF1):
                emit_proj_tile(pps, Q_t, "wq", xq_t, 1)
            with tc.tile_wait_until(KF0):
                emit_proj_tile(pps, K_t, "wk", xk_t, 0)
            with tc.tile_wait_until(VF0):
                emit_vt_quad(pps, 0)

        with (
            tc.tile_pool(name="lzps", bufs=2, space="PSUM") as lzps,
            tc.tile_pool(name="epool", bufs=4) as epool,
            tc.tile_pool(name="treep", bufs=2) as treep,
            tc.tile_pool(name="fold", bufs=3) as foldp,
            tc.tile_pool(name="opool", bufs=2) as opool,
        ):
            pairs = [(qt, j) for qt in range(2) for j in range(KC)]
            ps_tiles = {}

            def emit_qk(qt, j):
                ps = spool.tile([128, QT], f32, tag="ps", name="ps")
                for qq in range(2):
                    nc.tensor.matmul(
                        ps[:, qq * 512 : (qq + 1) * 512],
                        K_t[:, j * 128 : (j + 1) * 128],
                        Q_t[:, qt * QT + qq * 512 : qt * QT + (qq + 1) * 512],
                        start=True,
                        stop=True,
                    )
                ps_tiles[(qt, j)] = ps

            pc = None
            es_dt = None
            acc = None
            nfold = None
            emit_qk(*pairs[0])
            for i, (qt, j) in enumerate(pairs):
                if i + 1 < len(pairs):
                    emit_qk(*pairs[i + 1])
                # interleave remaining projections into the early iterations
                if qt == 0 and 1 <= j <= 14:
                    if j % 2 == 1:
                        t = (j + 1) // 2
                        with tc.tile_wait_until(K_FLOOR[t]):
                            emit_proj_tile(lzps, K_t, "wk", xk_t, t)
                    else:
                        q = j // 2
                        with tc.tile_wait_until(V_FLOOR[q]):
                            emit_vt_quad(lzps, q)
                if qt == 0 and 15 <= j <= 16:
                    with tc.tile_wait_until(XQ1):
                        emit_proj_tile(lzps, Q_t, "wq", xq_t, j - 13)
                if j == 0:
                    pc = pcpool.tile([128, QT], f32, tag="pc", name="pc")
                ps = ps_tiles.pop((qt, j))
                if j % 2 == 0:
                    es_dt = epool.tile([128, 2 * QT], bf16, tag="es", name="es")
                es = es_dt[:, (j % 2) * QT : (j % 2 + 1) * QT]
                nc.scalar.activation(es, ps[:], Exp)
                first, last = j == 0, j == KC - 1
                for qq in range(2):
                    sl = slice(qq * 512, (qq + 1) * 512)
                    nc.tensor.matmul(
                        pc[:, sl],
                        VT[:, j * 128 : (j + 1) * 128],
                        es[:, qq * 512 : (qq + 1) * 512],
                        start=first,
                        stop=last,
                    )
                # Softmax partial sums on DVE. Groups 0-2 (j 0..23): wide
                # [128, 2048] chained adds over 4 double-tiles, then one fold
                # to [128, QT]. Group 3 (j 24..31): per-double-tile narrow
                # folds chained, with the last fold shipped as its own
                # partial so the post-last-exp tail is just fold+DMA.
                if j % 2 == 1:
                    g, m = j // 8, (j % 8) // 2
                    if g < 3:
                        if m == 0:
                            acc = es_dt
                        else:
                            if m == 1:
                                nacc = treep.tile(
                                    [128, 2 * QT], bf16, tag="acc", name="acc"
                                )
                                nc.vector.tensor_add(nacc[:], acc[:], es_dt[:])
                                acc = nacc
                            else:
                                nc.vector.tensor_add(acc[:], acc[:], es_dt[:])
                            if m == 3:
                                fo = foldp.tile(
                                    [128, QT], bf16, tag="fo", name="fo"
                                )
                                nc.vector.tensor_add(
                                    fo[:], acc[:, 0:QT], acc[:, QT : 2 * QT]
                                )
                                nc.sync.dma_start(oss_d[qt, g], fo[:])
                    else:
                        fo = foldp.tile([128, QT], bf16, tag="fo", name="fo")
                        nc.vector.tensor_add(
                            fo[:], es_dt[:, 0:QT], es_dt[:, QT : 2 * QT]
                        )
                        if m == 0:
                            nfold = fo
                        elif m < 3:
                            nc.vector.tensor_add(nfold[:], nfold[:], fo[:])
                            if m == 2:
                                nc.sync.dma_start(oss_d[qt, 3], nfold[:])
                        else:
                            nc.sync.dma_start(oss_d[qt, 4], fo[:])
                if last:
                    so = opool.tile([128, QT], f32, tag="so", name="so")
                    for qq in range(2):
                        sl = slice(qq * 512, (qq + 1) * 512)
                        nc.vector.tensor_copy(so[:, sl], pc[:, sl])
                        nc.sync.dma_start(oc_d[qt, :, sl], so[:, sl])

    nc.compile()
    return nc


def make_in_maps(x, Wq, Wk, Wv):
    x = np.ascontiguousarray(
        np.asarray(x, dtype=np.float32).reshape(B, C, N).astype(np.float16)
    )
    wt = {
        "wq": np.ascontiguousarray(np.asarray(Wq, np.float32).T.astype(np.float16)),
        "wk": np.ascontiguousarray(np.asarray(Wk, np.float32).T.astype(np.float16)),
        "wv": np.ascontiguousarray(np.asarray(Wv, np.float32).T.astype(np.float16)),
    }

    in_maps = []
    for core in range(8):
        b, h = core // 2, core % 2
        in_maps.append(
            {
                "xk": x[b],
                "xq": np.ascontiguousarray(x[b][:, h * MQ : (h + 1) * MQ]),
                **wt,
            }
        )
    return in_maps


def assemble_output(results):
    out = np.empty((B, VC, N), dtype=np.float32)
    for core, r in enumerate(results):
        b, h = core // 2, core % 2
        # oss: [2, NG, 128, QT] partial sums; reduce groups+partitions
        sums = r["oss"].astype(np.float32).sum(axis=(1, 2))[:, None, :]  # [2,1,QT]
        core_out = r["oc"] / sums                                        # [2,128,QT]
        out[b, :, h * MQ : (h + 1) * MQ] = np.concatenate(
            [core_out[0], core_out[1]], axis=1
        )
    return out.reshape(B, VC, H, W)


def _results_sane(results):
    for r in results:
        oc, oss = r["oc"], np.asarray(r["oss"], dtype=np.float32)
        if not (np.isfinite(oc).all() and np.isfinite(oss).all()):
            return False
        if oss.sum(axis=(1, 2)).min() <= 0.0:      # softmax denominators
            return False
    return True


def kernel(x, Wq, Wk, Wv):
    global _cached_nc
    from concourse.bass_utils import run_bass_kernel_spmd

    if _cached_nc is None:
        _cached_nc = _build()
    in_maps = make_in_maps(x, Wq, Wk, Wv)
    results = None
    for attempt in range(3):
        try:
            res = run_bass_kernel_spmd(
                _cached_nc, in_maps, core_ids=list(range(8))
            )
        except Exception:
            if attempt == 2:
                raise
            continue
        results = res.results
        if _results_sane(results):
            break
    return assemble_output(results)


# revision 13
# speedup vs baseline: 1.1156x; 1.0167x over previous
"""Trainium2 Bass kernel for nn_AttentionHead.

Computation (per batch b):
    Q = Wq @ x_b, K = Wk @ x_b, V = Wv @ x_b        (x_b: [C=256, N=4096])
    S = Q^T K   [N, N];  A = softmax_k(S)
    out_b = V @ A^T                                  ([VC=128, N])

Sharding: 8 cores = 4 batches x 2 query-halves. Each core computes K/V^T for
its full batch and Q for its 2048-query half; a flash-style loop over 32 key
chunks of 128 never materializes the full [4096, 4096] affinity.

Numerics: the host casts x and the weights to fp16 (halves input DMA and
runs every matmul at the full-rate 16-bit PE path while keeping ~10 mantissa
bits through the logits, accumulated in fp32 PSUM). exp tiles are bf16 (fp16
would overflow: logits reach ~19 un-normalized). Softmax denominators: exp
tiles are summed on DVE in 2048-wide grouped chains down to 5 partials per
query-half; the final cross-partition reduction and the normalization happen
on the host during unshard.
"""

import numpy as np

B, C, VC, H, W = 4, 256, 128, 64, 64
N = H * W            # keys per batch
MQ = N // 2          # queries per core
QT = 1024            # query tile (PSUM-sized)
KC = N // 128        # key chunks of 128
NG = 5               # softmax partial-sum tiles per query tile

_cached_nc = None


def _build():
    from contextlib import ExitStack

    import concourse.bacc as bacc
    import concourse.mybir as mybir
    import concourse.tile as tile

    f32 = mybir.dt.float32
    f16 = mybir.dt.float16
    bf16 = mybir.dt.bfloat16
    Exp = mybir.ActivationFunctionType.Exp

    nc = bacc.Bacc("TRN2", target_bir_lowering=False, debug=False, num_devices=8)

    xk_d = nc.dram_tensor("xk", [C, N], f16, kind="ExternalInput")
    xq_d = nc.dram_tensor("xq", [C, MQ], f16, kind="ExternalInput")
    w_d = {
        "wq": nc.dram_tensor("wq", [C, VC], f16, kind="ExternalInput"),
        "wk": nc.dram_tensor("wk", [C, VC], f16, kind="ExternalInput"),
        "wv": nc.dram_tensor("wv", [C, VC], f16, kind="ExternalInput"),
    }
    oc_d = nc.dram_tensor("oc", [2, 128, QT], f32, kind="ExternalOutput")
    oss_d = nc.dram_tensor("oss", [2, NG, 128, QT], bf16, kind="ExternalOutput")

    with tile.TileContext(nc) as tc, ExitStack() as ctx:
        persist = ctx.enter_context(tc.tile_pool(name="persist", bufs=1))
        wpool = ctx.enter_context(tc.tile_pool(name="w", bufs=1))
        xp = ctx.enter_context(tc.tile_pool(name="xp", bufs=1))

        wts = {
            (nm, cc): wpool.tile(
                [128, VC], f16, tag=f"{nm}{cc}", name=f"{nm}{cc}"
            )
            for nm in ("wq", "wk", "wv")
            for cc in range(2)
        }

        K_t = persist.tile([128, N], f16, tag="K")
        Q_t = persist.tile([128, MQ], f16, tag="Q")
        VT = persist.tile([128, KC * 128], bf16, tag="VT")

        xk_t = [
            xp.tile([128, N], f16, tag=f"xk{cc}", name=f"xk{cc}") for cc in range(2)
        ]
        xq_t = [
            xp.tile([128, MQ], f16, tag=f"xq{cc}", name=f"xq{cc}") for cc in range(2)
        ]

        # Input DMA: weights go on the scalar queue (ACT idles until the
        # first exp anyway); each x piece is striped cc0->sync / cc1->gpsimd
        # so both halves complete together at the combined HBM rate, in
        # consumption order. Per-engine dependency windows bound in-flight
        # pieces so early pieces finish early.
        from concourse.tile_rust import add_dep_helper

        _dmas = {}

        def _issue(eng, ename, dst_ap, src_ap):
            lst = _dmas.setdefault(ename, [])
            ins = eng.dma_start(dst_ap, src_ap)
            if len(lst) >= 4:
                add_dep_helper(ins.ins, lst[-4].ins, reason="dma window")
            lst.append(ins)

        def dma_piece(dst_t, src_d, c0, c1):
            for cc, (eng, ename) in enumerate(
                ((nc.sync, "s"), (nc.gpsimd, "g"))
            ):
                _issue(
                    eng, ename,
                    dst_t[cc][:, c0:c1],
                    src_d[cc * 128 : (cc + 1) * 128, c0:c1],
                )

        for nm in ("wq", "wk", "wv"):
            for cc in range(2):
                _issue(
                    nc.scalar, "a",
                    wts[(nm, cc)][:], w_d[nm][cc * 128 : (cc + 1) * 128, :],
                )
        dma_piece(xq_t, xq_d, 0, 512)
        dma_piece(xq_t, xq_d, 512, 1024)
        dma_piece(xk_t, xk_d, 0, 512)
        dma_piece(xk_t, xk_d, 512, 1024)
        dma_piece(xk_t, xk_d, 1024, 2048)
        dma_piece(xk_t, xk_d, 2048, 3072)
        dma_piece(xk_t, xk_d, 3072, 4096)
        dma_piece(xq_t, xq_d, 1024, 2048)

        def emit_proj_tile(pool, dst, wnm, xt, t):
            ps = pool.tile([128, 512], f32, tag="projps", name="ps")
            for cc in range(2):
                nc.tensor.matmul(
                    ps[:],
                    wts[(wnm, cc)][:],
                    xt[cc][:, t * 512 : (t + 1) * 512],
                    start=(cc == 0),
                    stop=(cc == 1),
                )
            nc.vector.tensor_copy(dst[:, t * 512 : (t + 1) * 512], ps[:])

        def emit_vt_quad(pool, q):
            # V^T blocks 4q..4q+3: each [n=128, d=128] = x_block.T @ Wv.T;
            # four blocks share one PSUM tile so one wide copy drains them.
            tp = pool.tile([128, 512], f32, tag="projps", name="tp")
            for jj in range(4):
                j = 4 * q + jj
                for cc in range(2):
                    nc.tensor.matmul(
                        tp[:, jj * 128 : (jj + 1) * 128],
                        xk_t[cc][:, j * 128 : (j + 1) * 128],
                        wts[("wv", cc)][:],
                        start=(cc == 0),
                        stop=(cc == 1),
                    )
            nc.vector.tensor_copy(VT[:, q * 512 : (q + 1) * 512], tp[:])

        # Floors (ms): don't emit projection work before its DMA piece can
        # have landed (input streams at ~0.24 MB/us from ~7.6us), so the
        # in-order PE queue never blocks on a DMA semaphore.
        QF0, QF1, KF0, VF0 = 0.0091, 0.0102, 0.0112, 0.0118
        K_FLOOR = {1: 0.0126, 2: 0.0146, 3: 0.0151, 4: 0.0167, 5: 0.0172,
                   6: 0.0187, 7: 0.0192}
        V_FLOOR = {0: VF0, 1: 0.0129, 2: 0.0149, 3: 0.0154, 4: 0.0170,
                   5: 0.0175, 6: 0.0190, 7: 0.0195}
        XQ1 = 0.0209

        spool = ctx.enter_context(tc.tile_pool(name="spool", bufs=2, space="PSUM"))
        pcpool = ctx.enter_context(tc.tile_pool(name="pcpool", bufs=1, space="PSUM"))

        with tc.tile_pool(name="projps", bufs=2, space="PSUM") as pps:
            with tc.tile_wait_until(QF0):
                emit_proj_tile(pps, Q_t, "wq", xq_t, 0)
            with tc.tile_wait_until(QF1):
                emit_proj_tile(pps, Q_t, "wq", xq_t, 1)
            with tc.tile_wait_until(KF0):
                emit_proj_tile(pps, K_t, "wk", xk_t, 0)

        with (
            tc.tile_pool(name="lzps", bufs=2, space="PSUM") as lzps,
            tc.tile_pool(name="epool", bufs=4) as epool,
            tc.tile_pool(name="treep", bufs=2) as treep,
            tc.tile_pool(name="fold", bufs=3) as foldp,
            tc.tile_pool(name="opool", bufs=2) as opool,
        ):
            pairs = [(qt, j) for qt in range(2) for j in range(KC)]
            ps_tiles = {}

            def emit_qk(qt, j):
                ps = spool.tile([128, QT], f32, tag="ps", name="ps")
                for qq in range(2):
                    nc.tensor.matmul(
                        ps[:, qq * 512 : (qq + 1) * 512],
                        K_t[:, j * 128 : (j + 1) * 128],
                        Q_t[:, qt * QT + qq * 512 : qt * QT + (qq + 1) * 512],
                        start=True,
                        stop=True,
                    )
                ps_tiles[(qt, j)] = ps

            pc = None
            es_dt = None
            acc = None
            nfold = None
            emit_qk(*pairs[0])
            for i, (qt, j) in enumerate(pairs):
                if i + 1 < len(pairs):
                    emit_qk(*pairs[i + 1])
                # interleave remaining projections into the early iterations
                if qt == 0 and 0 <= j <= 14:
                    if j % 2 == 1:
                        t = (j + 1) // 2
                        with tc.tile_wait_until(K_FLOOR[t]):
                            emit_proj_tile(lzps, K_t, "wk", xk_t, t)
                    else:
                        q = j // 2
                        with tc.tile_wait_until(V_FLOOR[q]):
                            emit_vt_quad(lzps, q)
                if qt == 0 and 15 <= j <= 16:
                    with tc.tile_wait_until(XQ1):
                        emit_proj_tile(lzps, Q_t, "wq", xq_t, j - 13)
                if j == 0:
                    pc = pcpool.tile([128, QT], f32, tag="pc", name="pc")
                ps = ps_tiles.pop((qt, j))
                if j % 2 == 0:
                    es_dt = epool.tile([128, 2 * QT], bf16, tag="es", name="es")
                es = es_dt[:, (j % 2) * QT : (j % 2 + 1) * QT]
                nc.scalar.activation(es, ps[:], Exp)
                first, last = j == 0, j == KC - 1
                for qq in range(2):
                    sl = slice(qq * 512, (qq + 1) * 512)
                    nc.tensor.matmul(
                        pc[:, sl],
                        VT[:, j * 128 : (j + 1) * 128],
                        es[:, qq * 512 : (qq + 1) * 512],
                        start=first,
                        stop=last,
                    )
                # Softmax partial sums on DVE. Groups 0-2 (j 0..23): wide
                # [128, 2048] chained adds over 4 double-tiles, then one fold
                # to [128, QT]. Group 3 (j 24..31): per-double-tile narrow
                # folds chained, with the last fold shipped as its own
                # partial so the post-last-exp tail is just fold+DMA.
                if j % 2 == 1:
                    g, m = j // 8, (j % 8) // 2
                    if g < 3:
                        if m == 0:
                            acc = es_dt
                        else:
                            if m == 1:
                                nacc = treep.tile(
                                    [128, 2 * QT], bf16, tag="acc", name="acc"
                                )
                                nc.vector.tensor_add(nacc[:], acc[:], es_dt[:])
                                acc = nacc
                            else:
                                nc.vector.tensor_add(acc[:], acc[:], es_dt[:])
                            if m == 3:
                                fo = foldp.tile(
                                    [128, QT], bf16, tag="fo", name="fo"
                                )
                                nc.vector.tensor_add(
                                    fo[:], acc[:, 0:QT], acc[:, QT : 2 * QT]
                                )
                                nc.sync.dma_start(oss_d[qt, g], fo[:])
                    else:
                        fo = foldp.tile([128, QT], bf16, tag="fo", name="fo")
                        nc.vector.tensor_add(
                            fo[:], es_dt[:, 0:QT], es_dt[:, QT : 2 * QT]
                        )
                        if m == 0:
                            nfold = fo
                        elif m < 3:
                            nc.vector.tensor_add(nfold[:], nfold[:], fo[:])
                            if m == 2:
                                nc.sync.dma_start(oss_d[qt, 3], nfold[:])
                        else:
                            nc.sync.dma_start(oss_d[qt, 4], fo[:])
                if last:
                    # oc drain: qt=0 copies on DVE (slack mid-run); qt=1
                    # copies on ACT, which is idle once its last exp is done,
                    # so the tail never serializes with the DVE fold chain.
                    # The DMA rides gpsimd's queue, away from sync's oss DMAs.
                    so = opool.tile([128, QT], f32, tag="so", name="so")
                    for qq in range(2):
                        sl = slice(qq * 512, (qq + 1) * 512)
                        if qt == 0:
                            nc.vector.tensor_copy(so[:, sl], pc[:, sl])
                        else:
                            nc.scalar.copy(so[:, sl], pc[:, sl])
                        nc.gpsimd.dma_start(oc_d[qt, :, sl], so[:, sl])

    nc.compile()
    return nc


def make_in_maps(x, Wq, Wk, Wv):
    x = np.ascontiguousarray(
        np.asarray(x, dtype=np.float32).reshape(B, C, N).astype(np.float16)
    )
    wt = {
        "wq": np.ascontiguousarray(np.asarray(Wq, np.float32).T.astype(np.float16)),
        "wk": np.ascontiguousarray(np.asarray(Wk, np.float32).T.astype(np.float16)),
        "wv": np.ascontiguousarray(np.asarray(Wv, np.float32).T.astype(np.float16)),
    }

    in_maps = []
    for core in range(8):
        b, h = core // 2, core % 2
        in_maps.append(
            {
                "xk": x[b],
                "xq": np.ascontiguousarray(x[b][:, h * MQ : (h + 1) * MQ]),
                **wt,
            }
        )
    return in_maps


def assemble_output(results):
    out = np.empty((B, VC, N), dtype=np.float32)
    for core, r in enumerate(results):
        b, h = core // 2, core % 2
        # oss: [2, NG, 128, QT] partial sums; reduce groups+partitions
        sums = r["oss"].astype(np.float32).sum(axis=(1, 2))[:, None, :]  # [2,1,QT]
        core_out = r["oc"] / sums                                        # [2,128,QT]
        out[b, :, h * MQ : (h + 1) * MQ] = np.concatenate(
            [core_out[0], core_out[1]], axis=1
        )
    return out.reshape(B, VC, H, W)


def _results_sane(results):
    for r in results:
        oc, oss = r["oc"], np.asarray(r["oss"], dtype=np.float32)
        if not (np.isfinite(oc).all() and np.isfinite(oss).all()):
            return False
        if oss.sum(axis=(1, 2)).min() <= 0.0:      # softmax denominators
            return False
    return True


def kernel(x, Wq, Wk, Wv):
    global _cached_nc
    from concourse.bass_utils import run_bass_kernel_spmd

    if _cached_nc is None:
        _cached_nc = _build()
    in_maps = make_in_maps(x, Wq, Wk, Wv)
    results = None
    for attempt in range(3):
        try:
            res = run_bass_kernel_spmd(
                _cached_nc, in_maps, core_ids=list(range(8))
            )
        except Exception:
            if attempt == 2:
                raise
            continue
        results = res.results
        if _results_sane(results):
            break
    return assemble_output(results)


# revision 16
# speedup vs baseline: 1.1207x; 1.0046x over previous
"""Trainium2 Bass kernel for nn_AttentionHead.

Computation (per batch b):
    Q = Wq @ x_b, K = Wk @ x_b, V = Wv @ x_b        (x_b: [C=256, N=4096])
    S = Q^T K   [N, N];  A = softmax_k(S)
    out_b = V @ A^T                                  ([VC=128, N])

Sharding: 8 cores = 4 batches x 2 query-halves. Each core computes K/V^T for
its full batch and Q for its 2048-query half; a flash-style loop over 32 key
chunks of 128 never materializes the full [4096, 4096] affinity.

Numerics: the host casts x and the weights to fp16 (halves input DMA and
runs every matmul at the full-rate 16-bit PE path while keeping ~10 mantissa
bits through the logits, accumulated in fp32 PSUM). exp tiles are bf16 (fp16
would overflow: logits reach ~19 un-normalized). Softmax denominators: exp
tiles are summed on DVE in 2048-wide grouped chains down to 5 partials per
query-half; the final cross-partition reduction and the normalization happen
on the host during unshard.
"""

import numpy as np

B, C, VC, H, W = 4, 256, 128, 64, 64
N = H * W            # keys per batch
MQ = N // 2          # queries per core
QT = 1024            # query tile (PSUM-sized)
KC = N // 128        # key chunks of 128
NG = 5               # softmax partial-sum tiles per query tile

_cached_nc = None


def _build():
    from contextlib import ExitStack

    import concourse.bacc as bacc
    import concourse.mybir as mybir
    import concourse.tile as tile

    f32 = mybir.dt.float32
    f16 = mybir.dt.float16
    bf16 = mybir.dt.bfloat16
    Exp = mybir.ActivationFunctionType.Exp

    nc = bacc.Bacc("TRN2", target_bir_lowering=False, debug=False, num_devices=8)

    xk_d = nc.dram_tensor("xk", [C, N], f16, kind="ExternalInput")
    xq_d = nc.dram_tensor("xq", [C, MQ], f16, kind="ExternalInput")
    w_d = {
        "wq": nc.dram_tensor("wq", [C, VC], f16, kind="ExternalInput"),
        "wk": nc.dram_tensor("wk", [C, VC], f16, kind="ExternalInput"),
        "wv": nc.dram_tensor("wv", [C, VC], f16, kind="ExternalInput"),
    }
    oc_d = nc.dram_tensor("oc", [2, 128, QT], f32, kind="ExternalOutput")
    oss_d = nc.dram_tensor("oss", [2, NG, 128, QT], bf16, kind="ExternalOutput")

    with tile.TileContext(nc) as tc, ExitStack() as ctx:
        persist = ctx.enter_context(tc.tile_pool(name="persist", bufs=1))
        wpool = ctx.enter_context(tc.tile_pool(name="w", bufs=1))
        xp = ctx.enter_context(tc.tile_pool(name="xp", bufs=1))

        wts = {
            (nm, cc): wpool.tile(
                [128, VC], f16, tag=f"{nm}{cc}", name=f"{nm}{cc}"
            )
            for nm in ("wq", "wk", "wv")
            for cc in range(2)
        }

        K_t = persist.tile([128, N], f16, tag="K")
        Q_t = persist.tile([128, MQ], f16, tag="Q")
        VT = persist.tile([128, KC * 128], bf16, tag="VT")

        xk_t = [
            xp.tile([128, N], f16, tag=f"xk{cc}", name=f"xk{cc}") for cc in range(2)
        ]
        xq_t = [
            xp.tile([128, MQ], f16, tag=f"xq{cc}", name=f"xq{cc}") for cc in range(2)
        ]

        # Input DMA: weights go on the scalar queue (ACT idles until the
        # first exp anyway); each x piece is striped cc0->sync / cc1->gpsimd
        # so both halves complete together at the combined HBM rate, in
        # consumption order. Per-engine dependency windows bound in-flight
        # pieces so early pieces finish early.
        from concourse.tile_rust import add_dep_helper

        _dmas = {}

        def _issue(eng, ename, dst_ap, src_ap):
            lst = _dmas.setdefault(ename, [])
            ins = eng.dma_start(dst_ap, src_ap)
            if len(lst) >= 4:
                add_dep_helper(ins.ins, lst[-4].ins, reason="dma window")
            lst.append(ins)

        def dma_piece(dst_t, src_d, c0, c1):
            for cc, (eng, ename) in enumerate(
                ((nc.sync, "s"), (nc.gpsimd, "g"))
            ):
                _issue(
                    eng, ename,
                    dst_t[cc][:, c0:c1],
                    src_d[cc * 128 : (cc + 1) * 128, c0:c1],
                )

        for nm in ("wq", "wk", "wv"):
            for cc, (eng, ename) in enumerate(((nc.sync, "s"), (nc.gpsimd, "g"))):
                _issue(
                    eng, ename,
                    wts[(nm, cc)][:], w_d[nm][cc * 128 : (cc + 1) * 128, :],
                )
        dma_piece(xq_t, xq_d, 0, 512)
        dma_piece(xq_t, xq_d, 512, 1024)
        dma_piece(xk_t, xk_d, 0, 512)
        dma_piece(xk_t, xk_d, 512, 1024)
        dma_piece(xk_t, xk_d, 1024, 2048)
        dma_piece(xk_t, xk_d, 2048, 3072)
        dma_piece(xk_t, xk_d, 3072, 4096)
        dma_piece(xq_t, xq_d, 1024, 2048)

        def emit_proj_tile(pool, dst, wnm, xt, t):
            ps = pool.tile([128, 512], f32, tag="projps", name="ps")
            for cc in range(2):
                nc.tensor.matmul(
                    ps[:],
                    wts[(wnm, cc)][:],
                    xt[cc][:, t * 512 : (t + 1) * 512],
                    start=(cc == 0),
                    stop=(cc == 1),
                )
            nc.vector.tensor_copy(dst[:, t * 512 : (t + 1) * 512], ps[:])

        def emit_vt_quad(pool, q):
            # V^T blocks 4q..4q+3: each [n=128, d=128] = x_block.T @ Wv.T;
            # four blocks share one PSUM tile so one wide copy drains them.
            tp = pool.tile([128, 512], f32, tag="projps", name="tp")
            for jj in range(4):
                j = 4 * q + jj
                for cc in range(2):
                    nc.tensor.matmul(
                        tp[:, jj * 128 : (jj + 1) * 128],
                        xk_t[cc][:, j * 128 : (j + 1) * 128],
                        wts[("wv", cc)][:],
                        start=(cc == 0),
                        stop=(cc == 1),
                    )
            nc.vector.tensor_copy(VT[:, q * 512 : (q + 1) * 512], tp[:])

        # Floors (ms): don't emit projection work before its DMA piece can
        # have landed (input streams at ~0.24 MB/us from ~7.6us), so the
        # in-order PE queue never blocks on a DMA semaphore.
        QF0, QF1, KF0 = 0.0104, 0.0113, 0.0122
        K_FLOOR = {1: 0.0131, 2: 0.0148, 3: 0.0152, 4: 0.0166, 5: 0.0170,
                   6: 0.0184, 7: 0.0188}
        V_FLOOR = {0: 0.0126, 1: 0.0133, 2: 0.0150, 3: 0.0154, 4: 0.0168,
                   5: 0.0172, 6: 0.0186, 7: 0.0190}
        XQ1 = 0.0202

        spool = ctx.enter_context(tc.tile_pool(name="spool", bufs=2, space="PSUM"))
        pcpool = ctx.enter_context(tc.tile_pool(name="pcpool", bufs=1, space="PSUM"))

        with tc.tile_pool(name="projps", bufs=2, space="PSUM") as pps:
            with tc.tile_wait_until(QF0):
                emit_proj_tile(pps, Q_t, "wq", xq_t, 0)
            with tc.tile_wait_until(QF1):
                emit_proj_tile(pps, Q_t, "wq", xq_t, 1)
            with tc.tile_wait_until(KF0):
                emit_proj_tile(pps, K_t, "wk", xk_t, 0)

        with (
            tc.tile_pool(name="lzps", bufs=2, space="PSUM") as lzps,
            tc.tile_pool(name="epool", bufs=4) as epool,
            tc.tile_pool(name="treep", bufs=2) as treep,
            tc.tile_pool(name="fold", bufs=3) as foldp,
            tc.tile_pool(name="opool", bufs=2) as opool,
        ):
            pairs = [(qt, j) for qt in range(2) for j in range(KC)]
            ps_tiles = {}

            def emit_qk(qt, j):
                ps = spool.tile([128, QT], f32, tag="ps", name="ps")
                for qq in range(2):
                    nc.tensor.matmul(
                        ps[:, qq * 512 : (qq + 1) * 512],
                        K_t[:, j * 128 : (j + 1) * 128],
                        Q_t[:, qt * QT + qq * 512 : qt * QT + (qq + 1) * 512],
                        start=True,
                        stop=True,
                    )
                ps_tiles[(qt, j)] = ps

            pc = None
            es_dt = None
            acc = None
            nfold = None
            emit_qk(*pairs[0])
            for i, (qt, j) in enumerate(pairs):
                if i + 1 < len(pairs):
                    emit_qk(*pairs[i + 1])
                if j == 0:
                    pc = pcpool.tile([128, QT], f32, tag="pc", name="pc")
                ps = ps_tiles.pop((qt, j))
                if j % 2 == 0:
                    es_dt = epool.tile([128, 2 * QT], bf16, tag="es", name="es")
                es = es_dt[:, (j % 2) * QT : (j % 2 + 1) * QT]
                nc.scalar.activation(es, ps[:], Exp)
                # interleave remaining projections into the early iterations
                # (after the exp emission so they don't inflate its PE wait)
                if qt == 0 and 0 <= j <= 14:
                    if j % 2 == 1:
                        t = (j + 1) // 2
                        with tc.tile_wait_until(K_FLOOR[t]):
                            emit_proj_tile(lzps, K_t, "wk", xk_t, t)
                    else:
                        q = j // 2
                        with tc.tile_wait_until(V_FLOOR[q]):
                            emit_vt_quad(lzps, q)
                if qt == 0 and 15 <= j <= 16:
                    with tc.tile_wait_until(XQ1):
                        emit_proj_tile(lzps, Q_t, "wq", xq_t, j - 13)
                first, last = j == 0, j == KC - 1
                for qq in range(2):
                    sl = slice(qq * 512, (qq + 1) * 512)
                    nc.tensor.matmul(
                        pc[:, sl],
                        VT[:, j * 128 : (j + 1) * 128],
                        es[:, qq * 512 : (qq + 1) * 512],
                        start=first,
                        stop=last,
                    )
                # Softmax partial sums on DVE. Groups 0-2 (j 0..23): wide
                # [128, 2048] chained adds over 4 double-tiles, then one fold
                # to [128, QT]. Group 3 (j 24..31): per-double-tile narrow
                # folds chained, with the last fold shipped as its own
                # partial so the post-last-exp tail is just fold+DMA.
                if j % 2 == 1:
                    g, m = j // 8, (j % 8) // 2
                    if g < 3:
                        if m == 0:
                            acc = es_dt
                        else:
                            if m == 1:
                                nacc = treep.tile(
                                    [128, 2 * QT], bf16, tag="acc", name="acc"
                                )
                                nc.vector.tensor_add(nacc[:], acc[:], es_dt[:])
                                acc = nacc
                            else:
                                nc.vector.tensor_add(acc[:], acc[:], es_dt[:])
                            if m == 3:
                                fo = foldp.tile(
                                    [128, QT], bf16, tag="fo", name="fo"
                                )
                                nc.vector.tensor_add(
                                    fo[:], acc[:, 0:QT], acc[:, QT : 2 * QT]
                                )
                                nc.sync.dma_start(oss_d[qt, g], fo[:])
                    else:
                        fo = foldp.tile([128, QT], bf16, tag="fo", name="fo")
                        nc.vector.tensor_add(
                            fo[:], es_dt[:, 0:QT], es_dt[:, QT : 2 * QT]
                        )
                        if m == 0:
                            nfold = fo
                        elif m < 3:
                            nc.vector.tensor_add(nfold[:], nfold[:], fo[:])
                            if m == 2:
                                nc.sync.dma_start(oss_d[qt, 3], nfold[:])
                        else:
                            nc.sync.dma_start(oss_d[qt, 4], fo[:])
                if last:
                    # oc drain: qt=0 copies on DVE (slack mid-run); qt=1
                    # copies on ACT, which is idle once its last exp is done,
                    # so the tail never serializes with the DVE fold chain.
                    # The DMA rides gpsimd's queue, away from sync's oss DMAs.
                    so = opool.tile([128, QT], f32, tag="so", name="so")
                    for qq in range(2):
                        sl = slice(qq * 512, (qq + 1) * 512)
                        if qt == 0:
                            nc.vector.tensor_copy(so[:, sl], pc[:, sl])
                        else:
                            nc.scalar.copy(so[:, sl], pc[:, sl])
                        nc.gpsimd.dma_start(oc_d[qt, :, sl], so[:, sl])

    nc.compile()
    return nc


def make_in_maps(x, Wq, Wk, Wv):
    x = np.ascontiguousarray(
        np.asarray(x, dtype=np.float32).reshape(B, C, N).astype(np.float16)
    )
    wt = {
        "wq": np.ascontiguousarray(np.asarray(Wq, np.float32).T.astype(np.float16)),
        "wk": np.ascontiguousarray(np.asarray(Wk, np.float32).T.astype(np.float16)),
        "wv": np.ascontiguousarray(np.asarray(Wv, np.float32).T.astype(np.float16)),
    }

    in_maps = []
    for core in range(8):
        b, h = core // 2, core % 2
        in_maps.append(
            {
                "xk": x[b],
                "xq": np.ascontiguousarray(x[b][:, h * MQ : (h + 1) * MQ]),
                **wt,
            }
        )
    return in_maps


def assemble_output(results):
    out = np.empty((B, VC, N), dtype=np.float32)
    for core, r in enumerate(results):
        b, h = core // 2, core % 2
        # oss: [2, NG, 128, QT] partial sums; reduce groups+partitions
        sums = r["oss"].astype(np.float32).sum(axis=(1, 2))[:, None, :]  # [2,1,QT]
        core_out = r["oc"] / sums                                        # [2,128,QT]
        out[b, :, h * MQ : (h + 1) * MQ] = np.concatenate(
            [core_out[0], core_out[1]], axis=1
        )
    return out.reshape(B, VC, H, W)


def _results_sane(results):
    for r in results:
        oc, oss = r["oc"], np.asarray(r["oss"], dtype=np.float32)
        if not (np.isfinite(oc).all() and np.isfinite(oss).all()):
            return False
        if oss.sum(axis=(1, 2)).min() <= 0.0:      # softmax denominators
            return False
    return True


def kernel(x, Wq, Wk, Wv):
    global _cached_nc
    from concourse.bass_utils import run_bass_kernel_spmd

    if _cached_nc is None:
        _cached_nc = _build()
    in_maps = make_in_maps(x, Wq, Wk, Wv)
    results = None
    for attempt in range(3):
        try:
            res = run_bass_kernel_spmd(
                _cached_nc, in_maps, core_ids=list(range(8))
            )
        except Exception:
            if attempt == 2:
                raise
            continue
        results = res.results
        if _results_sane(results):
            break
    return assemble_output(results)
